# revision 33
# baseline (speedup 1.0000x reference)
"""BFP (block floating point) activation quantization kernel for Trainium2.

Problem: x [32, 256, 56, 56] f32; per (batch, 32-channel block, h, w) the 32
channels share an exponent e = floor(log2(max |x|)); quantize each value to
q * 2^(e-2) with q = clip(round(x / 2^(e-2)), -7, 7)  (mantissa=3 bits).

Strategy (pure data-parallel over batch, 4 images per core on 8 cores):
  - The host permutes each core's shard into the SBUF layout
    [p 128][chunk][ci 32][j J_k] (p = img*32 + blk*4 + hwq), so every DMA
    is a fully linear transfer and every tree level is a contiguous flat
    slice (bf16 2x perf mode). Chunks are sized [56, 56, 112*5, 56, 56]:
    small ends shorten the pipeline ramp-in and drain (and keep the early
    chunks fed while the load stream is still arriving), big middles
    amortize per-instruction overhead.
  - Per chunk the engines split:
      ScalarE: w = |x| -> bf16 (the only full pass; exponent survives)
      Vector:  maxabs tree (bf16 max levels, flat APs; last level emits
               fp32 m32), then ONE 8-stage custom DVE op computes the
               quantized output in a single f32 pass straight from m32:
                   t   = bits(m32) & POS_INF  (= 2^e; hard-wired +Inf mask)
                   Mb  = t * 3145728          (= MAGIC * 2^(e-2), exact)
                   B   = t * 1.8              (= 7.2*scale; any bound in
                                               (6.5, 7.5)*scale is exact)
                   out = (clip(x, -B, B) + Mb) - Mb    -> fp8 e4m3
               (the f32 +Mb add RNE-rounds to the scale grid whose step is
               the ulp at Mb's binade; -Mb is exact by Sterbenz)
  - fp8 e4m3 output is exact (q has <= 3 mantissa bits, e in [-7,7] here)
    and halves store traffic; the host un-permutes/upconverts.
  - ALL DMA goes through the SP HWDGE ring: SWDGE (gpsimd) stores were
    measured to throttle SDMA engine 15 via descriptor-ring port
    contention, dribbling the tail of the load stream out ~12us late.
    Every load trigger issues before any store trigger (the Sync queue is
    in-order; a store trigger waiting on its producer must never block a
    load trigger), and every chunk has its own SBUF buffer so loads
    prefetch the full input. The last chunk runs the custom op and stores
    in ci-halves to cut the drain.
"""

import numpy as np

import concourse.bass as bass
import concourse.tile as tile
from concourse import bacc, mybir
from concourse import dve_ops as _DO
from concourse.bass_utils import run_bass_kernel_spmd
from concourse.dve_spec import (
    C0, C1, Bin, Leaf, Spec, Src0, Src1, Zero, lower, maxx, minn,
)
from concourse.dve_uop import AluOp, DveOpSpec, InpSel

F32 = mybir.dt.float32
BF16 = mybir.dt.bfloat16
FP8 = mybir.dt.float8e4

N_CORES = 8
B, C, H, W = 32, 256, 56, 56
HW = H * W            # 3136
BPC = B // N_CORES    # 4 images per core
CI = 32               # channels per block
POS = HW // 4         # 784 j-positions per partition
TOTF = CI * POS       # 25088 free elems per partition
CHUNK_J = [56, 56] + [112] * 5 + [56, 56]
CHUNK_O = [int(v) for v in np.cumsum([0] + CHUNK_J[:-1])]
NCHUNK = len(CHUNK_J)
MAGIC = 12582912.0    # 1.5 * 2**23: RNE round-to-integer magic for |v| < 2**22

_CACHE = {}


def _register_bfp_op():
    """Custom DVE op: the entire BFP quantization in one 8-stage pass.

        t   = in1 & 0x7F800000   (AND with the hard-wired +Inf source;
                                  in1 = maxabs > 0, so t = 2^e exactly)
        Mb  = t * s0             (s0 = 3145728 = MAGIC/4 -> MAGIC*2^(e-2))
        B   = t * s1             (s1 = 1.8 -> 7.2*2^(e-2); any bound in
                                  (6.5, 7.5)*scale gives exact clipping)
        out = (clip(in0, -B, B) + Mb) - Mb
    """
    name = "BFP_FUSED_QUANT"
    for op in _DO.OPS:
        if op.name == name:
            return op

    def _ref(in0, in1, s0, s1, imm2):
        # per-stage f32 rounding is load-bearing (the magic-add trick)
        f32 = np.float32
        tb = (in1.astype(f32).view(np.uint32) & 0x7F800000).view(f32)
        mb = (tb * f32(s0)).astype(f32)
        bb = (tb * f32(s1)).astype(f32)
        v = np.minimum(np.maximum(in0.astype(f32), -bb), bb).astype(f32)
        r = (v + mb).astype(f32)
        return (r - mb).astype(f32)

    texp = Bin(AluOp.BITWISE_AND, Src1, Leaf(InpSel.POS_INF))
    mb_n = texp * C0
    b_n = texp * C1
    spec = Spec(
        body=(minn(maxx(Src0, Zero - b_n), b_n) + mb_n) - mb_n,
        reference=_ref,
    )
    row = _DO._CUSTOM_DVE_ROW_BASE + len(_DO.OPS)
    shas = {
        ver: DveOpSpec(
            name=name, opcode=row, uops=lower(spec, ver=ver), rd1_en=True
        ).sha(ver)
        for ver in ("v3", "v4")
    }
    op = _DO.DveOp(name, spec, subdim=False, uops_sha=shas)
    _DO.OPS.append(op)
    _DO.CUSTOM_DVE_SPECS[name] = spec
    _DO._SUB_OPCODE_FOR_NAME[name] = row
    return op


_BFP_OP = _register_bfp_op()


def _flat(ap):
    return ap.rearrange("p a b -> p (a b)")


def _build_program():
    if "nc" in _CACHE:
        return _CACHE["nc"]
    nc = bacc.Bacc(
        "TRN2",
        target_bir_lowering=False,
        debug=False,
        enable_asserts=False,
        num_devices=N_CORES,
    )
    # flat, chunk-major: chunk k occupies a CONTIGUOUS 128*CI*J_k block
    # (contiguous HBM streams keep the SDMA engines at full row locality)
    xu = nc.dram_tensor("xu", [128 * TOTF], F32, kind="ExternalInput")
    yo = nc.dram_tensor("yo", [128 * TOTF], FP8, kind="ExternalOutput")

    with tile.TileContext(nc) as tc:
        with (
            tc.tile_pool(name="xp", bufs=6) as xp,
            tc.tile_pool(name="xps", bufs=2) as xps,
            tc.tile_pool(name="wp", bufs=3) as wp,
            tc.tile_pool(name="wps", bufs=2) as wps,
            tc.tile_pool(name="op", bufs=3) as op_,
            tc.tile_pool(name="ops", bufs=2) as ops_,
            tc.tile_pool(name="mp", bufs=3) as mp,
            tc.tile_pool(name="mps", bufs=2) as mps,
        ):
            xts, ws, m32s, ots = {}, {}, {}, {}
            abs_done = set()

            def _jk(k):
                # (J_k, flat base of chunk k's contiguous DRAM block)
                return CHUNK_J[k], 128 * CI * CHUNK_O[k]

            def emit_load(k):
                if k >= NCHUNK or k in xts:
                    return
                jk, base = _jk(k)
                pool = xp if jk == 112 else xps
                xts[k] = pool.tile([128, CI, jk], F32, name="xt", tag="xt")
                nc.sync.dma_start(
                    xts[k][:],
                    bass.AP(xu, base, [[CI * jk, 128], [1, CI * jk]]),
                )

            def emit_abs(k):
                # |x| -> bf16; only the exponent of the maxabs survives.
                if k >= NCHUNK or k in abs_done:
                    return
                abs_done.add(k)
                jk, _ = _jk(k)
                pool = wp if jk == 112 else wps
                ws[k] = pool.tile([128, CI, jk], BF16, name="w", tag="w")
                nc.scalar.activation(
                    ws[k][:], xts[k][:], mybir.ActivationFunctionType.Abs,
                )

            def emit_tree(k):
                # maxabs tree: bf16 max levels (flat APs, 2x mode); the
                # last level emits fp32 m32
                if k >= NCHUNK:
                    return
                w = ws[k]
                for wdt in (16, 8, 4, 2):
                    nc.vector.tensor_tensor(
                        out=_flat(w[:, 0:wdt, :]),
                        in0=_flat(w[:, 0:wdt, :]),
                        in1=_flat(w[:, wdt : 2 * wdt, :]),
                        op=mybir.AluOpType.max,
                    )
                jk, _ = _jk(k)
                pool = mp if jk == 112 else mps
                m32s[k] = pool.tile([128, jk], F32, name="m32", tag="m32")
                nc.vector.tensor_tensor(
                    out=m32s[k][:], in0=w[:, 0, :], in1=w[:, 1, :],
                    op=mybir.AluOpType.max,
                )

            def emit_quant(k, h=None):
                # the whole quantization in one custom DVE pass -> fp8
                if k >= NCHUNK:
                    return
                jk, base = _jk(k)
                if k not in ots:
                    pool = op_ if jk == 112 else ops_
                    ots[k] = pool.tile([128, CI, jk], FP8, name="ot", tag="ot")
                m32 = m32s[k]
                sl = slice(None) if h is None else slice(16 * h, 16 * (h + 1))
                cn = CI if h is None else 16
                nc.vector._custom_dve(
                    _BFP_OP,
                    out=ots[k][:, sl, :], in0=xts[k][:, sl, :],
                    in1=m32[:].unsqueeze(1).broadcast_to([128, cn, jk]),
                    s0=3145728.0, s1=1.8,
                )
                soff = base if h in (None, 0) else base + CI * jk // 2
                n = CI * jk if h is None else CI * jk // 2
                # HWDGE (sync ring): SWDGE stores were observed to throttle
                # SDMA engine 15 via descriptor-ring port contention, making
                # the last load's packets trickle in ~12us late. The sync
                # queue is empty once the load prefetch triggers are done,
                # so store triggers never block anything there.
                nc.sync.dma_start(
                    bass.AP(yo, soff, [[CI * jk, 128], [1, n]]),
                    ots[k][:, sl, :],
                )

            # prologue: every chunk has its own buffer — trigger ALL loads
            # up front so no store trigger can ever block a load trigger
            # in the in-order Sync queue
            for k in range(NCHUNK):
                emit_load(k)
            emit_abs(0)
            emit_abs(1)

            for k in range(NCHUNK):
                emit_abs(k + 1)
                emit_tree(k)
                # |x| of chunk k+2 queued on ScalarE before quant(k) runs
                # so ScalarE always stays a chunk ahead of the tree
                emit_abs(k + 2)
                if k == NCHUNK - 1:
                    # drain: ci-halves, each stored right after its half
                    emit_quant(k, h=0)
                    emit_quant(k, h=1)
                else:
                    emit_quant(k)

    nc.compile()
    _CACHE["nc"] = nc
    return nc


def _permute_in(shard):
    # shard [4, 256, 3136] f32 -> flat chunk-major [chunk][p 128][ci][j],
    # p = img*32 + blk*4 + hwq, hw = hwq*784 + chunk_off + j
    t = shard.reshape(BPC, 8, CI, 4, POS)
    t = np.ascontiguousarray(t.transpose(0, 1, 3, 2, 4))  # [img,blk,hwq,ci,pos]
    t = t.reshape(128, CI, POS)
    return np.concatenate(
        [
            np.ascontiguousarray(t[:, :, o : o + j]).reshape(-1)
            for o, j in zip(CHUNK_O, CHUNK_J)
        ]
    )


def _permute_out(y):
    # y flat chunk-major f32 -> [4, 256, 3136]
    q = np.empty((128, CI, POS), dtype=y.dtype)
    f = 0
    for o, j in zip(CHUNK_O, CHUNK_J):
        n = 128 * CI * j
        q[:, :, o : o + j] = y[f : f + n].reshape(128, CI, j)
        f += n
    q = q.reshape(BPC, 8, 4, CI, POS).transpose(0, 1, 3, 2, 4)
    return np.ascontiguousarray(q).reshape(BPC, C, HW)


def kernel(activations=None, mantissa=3, blk=32, **_unused):
    x = np.ascontiguousarray(np.asarray(activations), dtype=np.float32)
    assert x.shape == (B, C, H, W), x.shape
    assert int(mantissa) == 3 and int(blk) == 32, (mantissa, blk)

    nc = _build_program()
    xr = x.reshape(B, C, HW)
    in_maps = [
        {"xu": _permute_in(xr[c * BPC : (c + 1) * BPC])} for c in range(N_CORES)
    ]
    res = run_bass_kernel_spmd(nc, in_maps, list(range(N_CORES))).results
    out = np.concatenate(
        [
            _permute_out(np.asarray(res[c]["yo"]).astype(np.float32)).reshape(
                BPC, C, H, W
            )
            for c in range(N_CORES)
        ],
        axis=0,
    )
    return out


def run_traced(activations):
    """test.py helper: run with NTFF tracing, return (out, BassKernelResults)."""
    x = np.ascontiguousarray(np.asarray(activations), dtype=np.float32)
    nc = _build_program()
    xr = x.reshape(B, C, HW)
    in_maps = [
        {"xu": _permute_in(xr[c * BPC : (c + 1) * BPC])} for c in range(N_CORES)
    ]
    r = run_bass_kernel_spmd(nc, in_maps, list(range(N_CORES)), trace=True)
    out = np.concatenate(
        [
            _permute_out(np.asarray(r.results[c]["yo"]).astype(np.float32)).reshape(
                BPC, C, H, W
            )
            for c in range(N_CORES)
        ],
        axis=0,
    )
    return out, r
